# revision 12
# baseline (speedup 1.0000x reference)
"""Trainium2 Bass kernel for Longformer self-attention (B=2, S=4096, D=768, H=12, HD=64, W=256, G=32).

Sharding: 8 cores = 2 batches x 4 head-groups (3 heads each). Each core computes its
batch's projections restricted to its 192 output channels, runs banded + global
attention for its 3 heads, and returns an unnormalized transposed output
([3, 65, S]: rows 0-63 = head-dim, row 64 = softmax denominator z) plus the raw
global-query output; the host divides by z, transposes, and assembles.

On-chip layout (per core):
  qT/kT/kgT  : transposed projections, head-dim on partitions. Heads 0/1 share a
               [128, S] tile (rows 0-63 / 64-127); head 2 occupies rows 64-127 of a
               second tile so its matmuls sit in PE row-groups 2-3 and run
               concurrently with head-0 work (row-group packing).
  v/vg       : natural layout [128-part seq tiles, per-head 64 cols + ones column]
  band scores: computed transposed ([128 keys, 640-query span] per key block), so
               the softmax denominator comes free via the ones column in PV.
Matmul inputs bf16 (hidden_states pre-cast on host), fp32 PSUM/softmax.
"""
import numpy as np
import ml_dtypes

import concourse.bass as bass
import concourse.mybir as mybir
import concourse.tile as tile
from concourse import bacc
from concourse.bass_utils import run_bass_kernel_spmd

B, S, D, H, HD = 2, 4096, 768, 12, 64
W = 256
G = 32
SCALE = 1.0 / np.float32(np.sqrt(HD))
NEG = -60.0
KB = 128
NKB = S // KB     # 32
QSB = 512
NQSB = S // QSB   # 8
NKT = D // 128    # 6
NNT = S // 512    # 8

BF = mybir.dt.bfloat16
F32 = mybir.dt.float32
AF = mybir.ActivationFunctionType
bf16 = ml_dtypes.bfloat16

_cache = {}


def _span(kb):
    k0 = KB * kb
    qlo, qhi = max(0, k0 - 2 * KB), min(S, k0 + 3 * KB)
    return qlo, qhi, qlo - (k0 - 2 * KB), qhi - (k0 - 2 * KB)


def _build():
    nc = bacc.Bacc(None, target_bir_lowering=False)

    hsb = nc.declare_dram_parameter("hsb", [S, D], BF, isOutput=False)
    w_q = nc.declare_dram_parameter("w_q", [128, NKT, 192], BF, isOutput=False)
    w_k = nc.declare_dram_parameter("w_k", [128, NKT, 192], BF, isOutput=False)
    w_kg = nc.declare_dram_parameter("w_kg", [128, NKT, 192], BF, isOutput=False)
    w_qg = nc.declare_dram_parameter("w_qg", [128, NKT, 192], BF, isOutput=False)
    w_vvg = nc.declare_dram_parameter("w_vvg", [128, NKT, 384], BF, isOutput=False)
    bvvg_d = nc.declare_dram_parameter("bvvg", [1, 384], BF, isOutput=False)
    bias_d = nc.declare_dram_parameter("bias_t", [128, 8], F32, isOutput=False)
    masks_d = nc.declare_dram_parameter("masks", [128, 256], F32, isOutput=False)
    id96_d = nc.declare_dram_parameter("id96", [96, 96], BF, isOutput=False)
    out_d = nc.declare_dram_parameter("out", [3, 65, S], F32, isOutput=True)
    outg_d = nc.declare_dram_parameter("outg", [G, 3, 65], F32, isOutput=True)

    with tile.TileContext(nc) as tc:
        with tc.tile_pool(name="persist", bufs=1) as pp:
            masks_t = pp.tile([128, 256], F32)
            ones_t = pp.tile([1, 128], BF)
            nc.gpsimd.dma_start(masks_t[:], masks_d[:])
            nc.vector.memset(ones_t[:], 1.0)

            qT01 = pp.tile([128, S], BF)
            qT2 = pp.tile([128, S], BF)      # rows 64-127 used
            kT01 = pp.tile([128, S], BF)
            kT2 = pp.tile([128, S], BF)      # rows 64-127 used
            v_nat = pp.tile([128, NKB, 3, 65], BF)
            exp_sg = pp.tile([G, 3, S], BF)
            nc.vector.memset(v_nat[:, :, :, 64:65], 1.0)

            def sl(t01, t2, h):
                return t01[64 * h:64 * h + 64] if h < 2 else t2[64:128]

            with tc.tile_pool(name="ac", bufs=1) as ac:
                kgT01 = ac.tile([128, S], BF)
                kgT2 = ac.tile([128, S], BF)  # rows 64-127 used
                qgT01 = ac.tile([128, G], BF)
                qgT2 = ac.tile([128, G], BF)  # rows 64-127 used
                vg_nat = ac.tile([128, NKB, 3, 65], BF)
                probs_g = ac.tile([96, S], BF)
                pb_gT = ac.tile([128, NKB, 96], BF)
                id96_t = ac.tile([96, 96], BF)
                og_sb = ac.tile([G, 3, 65], F32)
                nc.vector.memset(vg_nat[:, :, :, 64:65], 1.0)

                # ---------------- Phase A: projections --------------------------
                with (
                    tc.tile_pool(name="aw", bufs=1) as aw,
                    tc.tile_pool(name="hst", bufs=3) as hstp,
                    tc.tile_pool(name="apsum", bufs=6, space="PSUM") as apsum,
                ):
                    wq_t = aw.tile([128, NKT, 192], BF)
                    wk_t = aw.tile([128, NKT, 192], BF)
                    wkg_t = aw.tile([128, NKT, 192], BF)
                    wqg_t = aw.tile([128, NKT, 192], BF)
                    wvvg_t = aw.tile([128, NKT, 384], BF)
                    bvvg_t = aw.tile([1, 384], BF)
                    bias_t = aw.tile([128, 8], F32)
                    nc.gpsimd.dma_start(wq_t[:], w_q[:])
                    nc.gpsimd.dma_start(wk_t[:], w_k[:])
                    nc.gpsimd.dma_start(wkg_t[:], w_kg[:])
                    nc.gpsimd.dma_start(wqg_t[:], w_qg[:])
                    nc.gpsimd.dma_start(wvvg_t[:], w_vvg[:])
                    nc.gpsimd.dma_start(bvvg_t[:], bvvg_d[:])
                    nc.gpsimd.dma_start(bias_t[:], bias_d[:])
                    nc.gpsimd.dma_start(id96_t[:], id96_d[:])

                    tproj = [
                        (wq_t, 0, 1, qT01, qT2),
                        (wk_t, 2, 3, kT01, kT2),
                        (wkg_t, 4, 5, kgT01, kgT2),
                    ]
                    for nt in range(NNT):
                        c0 = 512 * nt
                        hst = hstp.tile([128, NKT, 512], BF)
                        for kt in range(NKT):
                            eng = nc.sync
                            eng.dma_start(
                                out=hst[:, kt, :],
                                in_=hsb[c0:c0 + 512, 128 * kt:128 * kt + 128],
                                transpose=True,
                            )
                        for (wt, c01, c2, d01, d2) in tproj:
                            ps = apsum.tile([128, 512], F32, tag="pp")
                            for kt in range(NKT):
                                nc.tensor.matmul(ps[:], wt[:, kt, 0:128], hst[:, kt, :],
                                                 start=(kt == 0), stop=(kt == NKT - 1))
                            nc.vector.tensor_scalar_add(
                                d01[:, c0:c0 + 512], ps[:], bias_t[:, c01:c01 + 1])
                            ps2 = apsum.tile([128, 512], F32, tag="pp")
                            for kt in range(NKT):
                                nc.tensor.matmul(ps2[64:128, :], wt[:, kt, 128:192],
                                                 hst[:, kt, :],
                                                 start=(kt == 0), stop=(kt == NKT - 1))
                            nc.scalar.activation(
                                d2[64:128, c0:c0 + 512], ps2[64:128, :], AF.Identity,
                                bias=bias_t[64:128, c2:c2 + 1], scale=1.0)
                        for s4 in range(4):
                            sb = 4 * nt + s4
                            psv = apsum.tile([128, 384], F32, tag="pp")
                            for kt in range(NKT):
                                nc.tensor.matmul(psv[:], hst[:, kt, 128 * s4:128 * s4 + 128],
                                                 wvvg_t[:, kt, :],
                                                 start=(kt == 0), stop=False)
                            nc.tensor.matmul(psv[:], ones_t[:, 0:128], bvvg_t[:],
                                             start=False, stop=True)
                            nc.vector.tensor_copy(
                                v_nat[:, sb, :, 0:64],
                                psv[:, 0:192].rearrange("p (h e) -> p h e", h=3))
                            nc.scalar.copy(
                                vg_nat[:, sb, :, 0:64],
                                psv[:, 192:384].rearrange("p (h e) -> p h e", h=3))
                        if nt == 0:
                            psq = apsum.tile([128, G], F32, tag="pp")
                            for kt in range(NKT):
                                nc.tensor.matmul(psq[:], wqg_t[:, kt, 0:128],
                                                 hst[:, kt, 0:G],
                                                 start=(kt == 0), stop=(kt == NKT - 1))
                            nc.vector.tensor_scalar_add(qgT01[:], psq[:], bias_t[:, 6:7])
                            psq2 = apsum.tile([128, G], F32, tag="pp")
                            for kt in range(NKT):
                                nc.tensor.matmul(psq2[64:128, :], wqg_t[:, kt, 128:192],
                                                 hst[:, kt, 0:G],
                                                 start=(kt == 0), stop=(kt == NKT - 1))
                            nc.vector.tensor_scalar_add(qgT2[64:128, :], psq2[64:128, :],
                                                        bias_t[64:128, 7:8])

                # ------------- Phase B: exp of global-key scores ----------------
                with tc.tile_pool(name="bpsum", bufs=2, space="PSUM") as bpsum:
                    for c8 in range(NNT):
                        c0 = 512 * c8
                        ps = bpsum.tile([G, 3, 512], F32, tag="sg")
                        for h in range(3):
                            nc.tensor.matmul(ps[:, h, :],
                                             sl(kT01, kT2, h)[:, 0:G],
                                             sl(qT01, qT2, h)[:, c0:c0 + 512])
                        nc.scalar.activation(exp_sg[:, :, c0:c0 + 512], ps[:], AF.Exp)

                # ------------- Phase C: global-query attention ------------------
                with (
                    tc.tile_pool(name="cpsum", bufs=2, space="PSUM") as cpsum,
                    tc.tile_pool(name="cgps", bufs=3, space="PSUM") as cgps,
                ):
                    for c8 in range(NNT):
                        c0 = 512 * c8
                        ps = cpsum.tile([96, 512], F32, tag="qg")
                        for h in range(3):
                            nc.tensor.matmul(ps[32 * h:32 * h + 32, :],
                                             sl(qgT01, qgT2, h)[:],
                                             sl(kgT01, kgT2, h)[:, c0:c0 + 512])
                        nc.scalar.activation(probs_g[:, c0:c0 + 512], ps[:], AF.Exp)
                    for t in range(NKB):
                        pst = cpsum.tile([128, 96], BF, tag="qg")
                        nc.tensor.transpose(pst[:], probs_g[:, 128 * t:128 * t + 128],
                                            id96_t[:])
                        if t % 2 == 0:
                            nc.vector.tensor_copy(pb_gT[:, t, :], pst[:])
                        else:
                            nc.scalar.copy(pb_gT[:, t, :], pst[:])
                    for h in range(3):
                        go = cgps.tile([G, 65], F32, tag="go")
                        for t in range(NKB):
                            nc.tensor.matmul(go[:], pb_gT[:, t, 32 * h:32 * h + 32],
                                             vg_nat[:, t, h, :],
                                             start=(t == 0), stop=(t == NKB - 1))
                        nc.vector.tensor_copy(og_sb[:, h, :], go[:])
                    nc.sync.dma_start(outg_d[:], og_sb[:])

            # ------------- Phase D: banded attention ----------------------------
            with (
                tc.tile_pool(name="dstag", bufs=1) as dstag,
                tc.tile_pool(name="spsum", bufs=2, space="PSUM") as spsum,
                tc.tile_pool(name="opsum", bufs=2, space="PSUM") as opsum,
                tc.tile_pool(name="pbt", bufs=36) as pbtp,
            ):
                stag = [dstag.tile([65, S], F32, name=f"stag{h}") for h in range(3)]
                pbt = {}

                def do_kb(kb):
                    k0 = KB * kb
                    qlo, qhi, llo, lhi = _span(kb)
                    pieces = [(a, b) for (a, b) in
                              [(llo, min(lhi, 512)), (max(llo, 512), lhi)] if a < b]
                    tiles = []
                    for h in range(3):
                        ps = spsum.tile([128, 640], F32, tag="sc")
                        tiles.append(ps)
                    # interleave heads per piece so PE packs row-groups
                    for (a, b2) in pieces:
                        for h in range(3):
                            nc.tensor.matmul(
                                tiles[h][:, a:b2],
                                sl(kT01, kT2, h)[:, k0:k0 + KB],
                                sl(qT01, qT2, h)[:, qlo + (a - llo):qlo + (a - llo) + (b2 - a)])
                    for h in range(3):
                        ps = tiles[h]
                        if llo == 0:
                            nc.vector.tensor_add(ps[:, 0:KB], ps[:, 0:KB], masks_t[:, 0:KB])
                        if lhi == 5 * KB:
                            nc.vector.tensor_add(ps[:, 512:640], ps[:, 512:640],
                                                 masks_t[:, KB:2 * KB])
                        t_ = pbtp.tile([128, 640], BF, tag="pb")
                        nc.scalar.activation(t_[:, llo:lhi], ps[:, llo:lhi], AF.Exp)
                        pbt[(kb, h)] = t_

                for qs in range(NQSB):
                    q0 = QSB * qs
                    new_kbs = range(0, 6) if qs == 0 else range(4 * qs + 2,
                                                                min(NKB, 4 * qs + 6))
                    for kb in new_kbs:
                        do_kb(kb)
                    kbs = list(range(max(0, 4 * qs - 2), min(NKB, 4 * qs + 6)))
                    for h in range(3):
                        po = opsum.tile([65, 512], F32, tag="po")
                        nc.tensor.matmul(po[:], v_nat[0:G, 0, h, :],
                                         exp_sg[:, h, q0:q0 + 512],
                                         start=True, stop=False)
                        for i, kb in enumerate(kbs):
                            k0 = KB * kb
                            qlo, qhi, llo, lhi = _span(kb)
                            a, b2 = max(qlo, q0), min(qhi, q0 + QSB)
                            la = a - (k0 - 2 * KB)
                            nc.tensor.matmul(po[:, a - q0:b2 - q0], v_nat[:, kb, h, :],
                                             pbt[(kb, h)][:, la:la + (b2 - a)],
                                             start=False, stop=(i == len(kbs) - 1))
                        if h % 2 == 0:
                            nc.vector.tensor_copy(stag[h][:, q0:q0 + 512], po[:])
                        else:
                            nc.scalar.copy(stag[h][:, q0:q0 + 512], po[:])
                for h in range(3):
                    for a in range(2):
                        nc.sync.dma_start(out_d[h, :, 2048 * a:2048 * a + 2048],
                                          stag[h][:, 2048 * a:2048 * a + 2048])

    nc.compile()
    return nc


def _prep_inputs(inputs):
    hs = np.asarray(inputs["hidden_states"], dtype=np.float32)
    maps = []
    j = np.arange(KB)
    p = np.arange(KB)[:, None]
    m_lo = np.where(j[None, :] >= p, 0.0, NEG).astype(np.float32)
    m_hi = np.where(j[None, :] <= p, 0.0, NEG).astype(np.float32)
    masks = np.concatenate([m_lo, m_hi], axis=1)
    id96 = np.eye(96, dtype=bf16)

    def wtiles(w):
        n = w.shape[1]
        return np.ascontiguousarray(w.reshape(NKT, 128, n).transpose(1, 0, 2)).astype(bf16)

    for c in range(8):
        b, hg = c // 4, c % 4
        cols = slice(192 * hg, 192 * hg + 192)
        Wq = np.asarray(inputs["Wq"], np.float32)[:, cols] * SCALE
        bq = np.asarray(inputs["bq"], np.float32)[cols] * SCALE
        Wqg = np.asarray(inputs["Wqg"], np.float32)[:, cols] * SCALE
        bqg = np.asarray(inputs["bqg"], np.float32)[cols] * SCALE
        Wk = np.asarray(inputs["Wk"], np.float32)[:, cols]
        bk = np.asarray(inputs["bk"], np.float32)[cols]
        Wkg = np.asarray(inputs["Wkg"], np.float32)[:, cols]
        bkg = np.asarray(inputs["bkg"], np.float32)[cols]
        Wv = np.asarray(inputs["Wv"], np.float32)[:, cols]
        bv = np.asarray(inputs["bv"], np.float32)[cols]
        Wvg = np.asarray(inputs["Wvg"], np.float32)[:, cols]
        bvg = np.asarray(inputs["bvg"], np.float32)[cols]

        # m2 biases live at partitions 64-127 now
        bias_t2 = np.zeros((128, 8), np.float32)
        bias_t2[:, 0], bias_t2[64:128, 1] = bq[0:128], bq[128:192]
        bias_t2[:, 2], bias_t2[64:128, 3] = bk[0:128], bk[128:192]
        bias_t2[:, 4], bias_t2[64:128, 5] = bkg[0:128], bkg[128:192]
        bias_t2[:, 6], bias_t2[64:128, 7] = bqg[0:128], bqg[128:192]

        maps.append({
            "hsb": hs[b].astype(bf16),
            "w_q": wtiles(Wq),
            "w_k": wtiles(Wk),
            "w_kg": wtiles(Wkg),
            "w_qg": wtiles(Wqg),
            "w_vvg": wtiles(np.concatenate([Wv, Wvg], axis=1)),
            "bvvg": np.concatenate([bv, bvg])[None, :].astype(bf16),
            "bias_t": bias_t2,
            "masks": masks,
            "id96": id96,
        })
    return maps


def kernel(**inputs):
    g = int(np.asarray(inputs["num_global"]))
    assert g == G, f"kernel compiled for num_global=32, got {g}"
    if "nc" not in _cache:
        _cache["nc"] = _build()
    nc = _cache["nc"]
    in_maps = _prep_inputs(inputs)
    res = run_bass_kernel_spmd(nc, in_maps, list(range(8)))
    return assemble(res.results)


def assemble(results):
    out = np.zeros((B, S, D), np.float32)
    for c in range(8):
        b, hg = c // 4, c % 4
        o = results[c]["out"]          # [3, 65, S]
        og = results[c]["outg"]        # [G, 3, 65]
        for h in range(3):
            col = 192 * hg + 64 * h
            out[b, :, col:col + 64] = (o[h, 0:64] / o[h, 64]).T
            out[b, 0:G, col:col + 64] = og[:, h, 0:64] / og[:, h, 64:65]
    return out


# revision 13
# speedup vs baseline: 1.2003x; 1.2003x over previous
"""Trainium2 Bass kernel for Longformer self-attention (B=2, S=4096, D=768, H=12, HD=64, W=256, G=32).

Sharding: 8 cores = 2 batches x 4 head-groups (3 heads each). Each core computes its
batch's projections restricted to its 192 output channels, runs banded + global
attention for its 3 heads, and returns an unnormalized transposed output
([3, 65, S]: rows 0-63 = head-dim, row 64 = softmax denominator z) plus the raw
global-query output; the host divides by z, transposes, and assembles.

On-chip layout (per core):
  qT/kT/kgT  : transposed projections, head-dim on partitions. Heads 0/1 share a
               [128, S] tile (rows 0-63 / 64-127); head 2 occupies rows 64-127 of a
               second tile so its matmuls sit in PE row-groups 2-3 and run
               concurrently with head-0 work (row-group packing).
  v/vg       : natural layout [128-part seq tiles, per-head 64 cols + ones column]
  band scores: computed transposed ([128 keys, 640-query span] per key block), so
               the softmax denominator comes free via the ones column in PV.
Matmul inputs bf16 (hidden_states pre-cast on host), fp32 PSUM/softmax.
"""
import numpy as np
import ml_dtypes

import concourse.bass as bass
import concourse.mybir as mybir
import concourse.tile as tile
from concourse import bacc
from concourse.bass_utils import run_bass_kernel_spmd

B, S, D, H, HD = 2, 4096, 768, 12, 64
W = 256
G = 32
SCALE = 1.0 / np.float32(np.sqrt(HD))
NEG = -60.0
KB = 128
NKB = S // KB     # 32
QSB = 512
NQSB = S // QSB   # 8
NKT = D // 128    # 6
NNT = S // 512    # 8

BF = mybir.dt.bfloat16
F32 = mybir.dt.float32
AF = mybir.ActivationFunctionType
bf16 = ml_dtypes.bfloat16

_cache = {}


def _span(kb):
    k0 = KB * kb
    qlo, qhi = max(0, k0 - 2 * KB), min(S, k0 + 3 * KB)
    return qlo, qhi, qlo - (k0 - 2 * KB), qhi - (k0 - 2 * KB)


def _build():
    nc = bacc.Bacc(None, target_bir_lowering=False)

    hsb = nc.declare_dram_parameter("hsb", [S, D], BF, isOutput=False)
    w_q = nc.declare_dram_parameter("w_q", [128, NKT, 192], BF, isOutput=False)
    w_k = nc.declare_dram_parameter("w_k", [128, NKT, 192], BF, isOutput=False)
    w_kg = nc.declare_dram_parameter("w_kg", [128, NKT, 192], BF, isOutput=False)
    w_qg = nc.declare_dram_parameter("w_qg", [128, NKT, 192], BF, isOutput=False)
    w_vvg = nc.declare_dram_parameter("w_vvg", [128, NKT, 384], BF, isOutput=False)
    bvvg_d = nc.declare_dram_parameter("bvvg", [1, 384], BF, isOutput=False)
    bias_d = nc.declare_dram_parameter("bias_t", [128, 8], F32, isOutput=False)
    masks_d = nc.declare_dram_parameter("masks", [128, 256], F32, isOutput=False)
    id96_d = nc.declare_dram_parameter("id96", [96, 96], BF, isOutput=False)
    out_d = nc.declare_dram_parameter("out", [3, 65, S], F32, isOutput=True)
    outg_d = nc.declare_dram_parameter("outg", [G, 3, 65], F32, isOutput=True)

    with tile.TileContext(nc) as tc:
        with tc.tile_pool(name="persist", bufs=1) as pp:
            masks_t = pp.tile([128, 256], F32)
            ones_t = pp.tile([1, 128], BF)
            nc.sync.dma_start(masks_t[:], masks_d[:])
            nc.vector.memset(ones_t[:], 1.0)

            qT01 = pp.tile([128, S], BF)
            qT2 = pp.tile([128, S], BF)      # rows 64-127 used
            kT01 = pp.tile([128, S], BF)
            kT2 = pp.tile([128, S], BF)      # rows 64-127 used
            v_nat = pp.tile([128, NKB, 3, 65], BF)
            exp_sg = pp.tile([G, 3, S], BF)
            nc.vector.memset(v_nat[:, :, :, 64:65], 1.0)

            def sl(t01, t2, h):
                return t01[64 * h:64 * h + 64] if h < 2 else t2[64:128]

            with tc.tile_pool(name="ac", bufs=1) as ac:
                kgT01 = ac.tile([128, S], BF)
                kgT2 = ac.tile([128, S], BF)  # rows 64-127 used
                qgT01 = ac.tile([128, G], BF)
                qgT2 = ac.tile([128, G], BF)  # rows 64-127 used
                vg_nat = ac.tile([128, NKB, 3, 65], BF)
                probs_g = ac.tile([96, S], BF)
                pb_gT = ac.tile([128, NKB, 96], BF)
                id96_t = ac.tile([96, 96], BF)
                og_sb = ac.tile([G, 3, 65], F32)
                nc.vector.memset(vg_nat[:, :, :, 64:65], 1.0)

                # ---------------- Phase A: projections --------------------------
                with (
                    tc.tile_pool(name="aw", bufs=1) as aw,
                    tc.tile_pool(name="hst", bufs=4) as hstp,
                    tc.tile_pool(name="apsum", bufs=6, space="PSUM") as apsum,
                ):
                    wq_t = aw.tile([128, NKT, 192], BF)
                    wk_t = aw.tile([128, NKT, 192], BF)
                    wkg_t = aw.tile([128, NKT, 192], BF)
                    wqg_t = aw.tile([128, NKT, 192], BF)
                    wvvg_t = aw.tile([128, NKT, 384], BF)
                    bvvg_t = aw.tile([1, 384], BF)
                    bias_t = aw.tile([128, 8], F32)
                    nc.sync.dma_start(wq_t[:], w_q[:])
                    nc.sync.dma_start(wk_t[:], w_k[:])
                    nc.sync.dma_start(wkg_t[:], w_kg[:])
                    nc.sync.dma_start(wqg_t[:], w_qg[:])
                    nc.sync.dma_start(wvvg_t[:], w_vvg[:])
                    nc.sync.dma_start(bvvg_t[:], bvvg_d[:])
                    nc.sync.dma_start(bias_t[:], bias_d[:])
                    nc.sync.dma_start(id96_t[:], id96_d[:])

                    tproj = [
                        (wq_t, 0, 1, qT01, qT2),
                        (wk_t, 2, 3, kT01, kT2),
                        (wkg_t, 4, 5, kgT01, kgT2),
                    ]
                    for nt in range(NNT):
                        c0 = 512 * nt
                        hst = hstp.tile([128, NKT, 512], BF)
                        for kt in range(NKT):
                            eng = nc.sync
                            eng.dma_start(
                                out=hst[:, kt, :],
                                in_=hsb[c0:c0 + 512, 128 * kt:128 * kt + 128],
                                transpose=True,
                            )
                        for (wt, c01, c2, d01, d2) in tproj:
                            ps = apsum.tile([128, 512], F32, tag="pp")
                            for kt in range(NKT):
                                nc.tensor.matmul(ps[:], wt[:, kt, 0:128], hst[:, kt, :],
                                                 start=(kt == 0), stop=(kt == NKT - 1))
                            nc.vector.tensor_scalar_add(
                                d01[:, c0:c0 + 512], ps[:], bias_t[:, c01:c01 + 1])
                            ps2 = apsum.tile([128, 512], F32, tag="pp")
                            for kt in range(NKT):
                                nc.tensor.matmul(ps2[64:128, :], wt[:, kt, 128:192],
                                                 hst[:, kt, :],
                                                 start=(kt == 0), stop=(kt == NKT - 1))
                            nc.scalar.activation(
                                d2[64:128, c0:c0 + 512], ps2[64:128, :], AF.Identity,
                                bias=bias_t[64:128, c2:c2 + 1], scale=1.0)
                        for s4 in range(4):
                            sb = 4 * nt + s4
                            psv = apsum.tile([128, 384], F32, tag="pp")
                            for kt in range(NKT):
                                nc.tensor.matmul(psv[:], hst[:, kt, 128 * s4:128 * s4 + 128],
                                                 wvvg_t[:, kt, :],
                                                 start=(kt == 0), stop=False)
                            nc.tensor.matmul(psv[:], ones_t[:, 0:128], bvvg_t[:],
                                             start=False, stop=True)
                            nc.vector.tensor_copy(
                                v_nat[:, sb, :, 0:64],
                                psv[:, 0:192].rearrange("p (h e) -> p h e", h=3))
                            nc.scalar.copy(
                                vg_nat[:, sb, :, 0:64],
                                psv[:, 192:384].rearrange("p (h e) -> p h e", h=3))
                        if nt == 0:
                            psq = apsum.tile([128, G], F32, tag="pp")
                            for kt in range(NKT):
                                nc.tensor.matmul(psq[:], wqg_t[:, kt, 0:128],
                                                 hst[:, kt, 0:G],
                                                 start=(kt == 0), stop=(kt == NKT - 1))
                            nc.vector.tensor_scalar_add(qgT01[:], psq[:], bias_t[:, 6:7])
                            psq2 = apsum.tile([128, G], F32, tag="pp")
                            for kt in range(NKT):
                                nc.tensor.matmul(psq2[64:128, :], wqg_t[:, kt, 128:192],
                                                 hst[:, kt, 0:G],
                                                 start=(kt == 0), stop=(kt == NKT - 1))
                            nc.vector.tensor_scalar_add(qgT2[64:128, :], psq2[64:128, :],
                                                        bias_t[64:128, 7:8])

                # ------------- Phase B: exp of global-key scores ----------------
                with tc.tile_pool(name="bpsum", bufs=2, space="PSUM") as bpsum:
                    for c8 in range(NNT):
                        c0 = 512 * c8
                        ps = bpsum.tile([G, 3, 512], F32, tag="sg")
                        for h in range(3):
                            nc.tensor.matmul(ps[:, h, :],
                                             sl(kT01, kT2, h)[:, 0:G],
                                             sl(qT01, qT2, h)[:, c0:c0 + 512])
                        nc.scalar.activation(exp_sg[:, :, c0:c0 + 512], ps[:], AF.Exp)

                # ------------- Phase C: global-query attention ------------------
                with (
                    tc.tile_pool(name="cpsum", bufs=2, space="PSUM") as cpsum,
                    tc.tile_pool(name="cgps", bufs=3, space="PSUM") as cgps,
                ):
                    for c8 in range(NNT):
                        c0 = 512 * c8
                        ps = cpsum.tile([96, 512], F32, tag="qg")
                        for h in range(3):
                            nc.tensor.matmul(ps[32 * h:32 * h + 32, :],
                                             sl(qgT01, qgT2, h)[:],
                                             sl(kgT01, kgT2, h)[:, c0:c0 + 512])
                        nc.scalar.activation(probs_g[:, c0:c0 + 512], ps[:], AF.Exp)
                    for t in range(NKB):
                        pst = cpsum.tile([128, 96], BF, tag="qg")
                        nc.tensor.transpose(pst[:], probs_g[:, 128 * t:128 * t + 128],
                                            id96_t[:])
                        if t % 2 == 0:
                            nc.vector.tensor_copy(pb_gT[:, t, :], pst[:])
                        else:
                            nc.scalar.copy(pb_gT[:, t, :], pst[:])
                    for h in range(3):
                        go = cgps.tile([G, 65], F32, tag="go")
                        for t in range(NKB):
                            nc.tensor.matmul(go[:], pb_gT[:, t, 32 * h:32 * h + 32],
                                             vg_nat[:, t, h, :],
                                             start=(t == 0), stop=(t == NKB - 1))
                        nc.vector.tensor_copy(og_sb[:, h, :], go[:])
                    nc.sync.dma_start(outg_d[:], og_sb[:])

            # ------------- Phase D: banded attention ----------------------------
            with (
                tc.tile_pool(name="dstag", bufs=1) as dstag,
                tc.tile_pool(name="spsum", bufs=3, space="PSUM") as spsum,
                tc.tile_pool(name="opsum", bufs=2, space="PSUM") as opsum,
                tc.tile_pool(name="pbt", bufs=36) as pbtp,
            ):
                stag = [dstag.tile([65, S], F32, name=f"stag{h}") for h in range(3)]
                pbt = {}

                def do_kb(kb):
                    k0 = KB * kb
                    qlo, qhi, llo, lhi = _span(kb)
                    pieces = [(a, b) for (a, b) in
                              [(llo, min(lhi, 512)), (max(llo, 512), lhi)] if a < b]
                    tiles = []
                    for h in range(3):
                        ps = spsum.tile([128, 640], F32, tag="sc")
                        tiles.append(ps)
                    # interleave heads per piece so PE packs row-groups
                    for (a, b2) in pieces:
                        for h in range(3):
                            nc.tensor.matmul(
                                tiles[h][:, a:b2],
                                sl(kT01, kT2, h)[:, k0:k0 + KB],
                                sl(qT01, qT2, h)[:, qlo + (a - llo):qlo + (a - llo) + (b2 - a)])
                    for h in range(3):
                        ps = tiles[h]
                        if llo == 0:
                            nc.vector.tensor_add(ps[:, 0:KB], ps[:, 0:KB], masks_t[:, 0:KB])
                        if lhi == 5 * KB:
                            nc.vector.tensor_add(ps[:, 512:640], ps[:, 512:640],
                                                 masks_t[:, KB:2 * KB])
                        t_ = pbtp.tile([128, 640], BF, tag="pb")
                        nc.scalar.activation(t_[:, llo:lhi], ps[:, llo:lhi], AF.Exp)
                        pbt[(kb, h)] = t_

                for qs in range(NQSB):
                    q0 = QSB * qs
                    new_kbs = range(0, 6) if qs == 0 else range(4 * qs + 2,
                                                                min(NKB, 4 * qs + 6))
                    for kb in new_kbs:
                        do_kb(kb)
                    kbs = list(range(max(0, 4 * qs - 2), min(NKB, 4 * qs + 6)))
                    for h in range(3):
                        po = opsum.tile([65, 512], F32, tag="po")
                        nc.tensor.matmul(po[:], v_nat[0:G, 0, h, :],
                                         exp_sg[:, h, q0:q0 + 512],
                                         start=True, stop=False)
                        for i, kb in enumerate(kbs):
                            k0 = KB * kb
                            qlo, qhi, llo, lhi = _span(kb)
                            a, b2 = max(qlo, q0), min(qhi, q0 + QSB)
                            la = a - (k0 - 2 * KB)
                            nc.tensor.matmul(po[:, a - q0:b2 - q0], v_nat[:, kb, h, :],
                                             pbt[(kb, h)][:, la:la + (b2 - a)],
                                             start=False, stop=(i == len(kbs) - 1))
                        if h % 2 == 0:
                            nc.vector.tensor_copy(stag[h][:, q0:q0 + 512], po[:])
                        else:
                            nc.scalar.copy(stag[h][:, q0:q0 + 512], po[:])
                for h in range(3):
                    for a in range(2):
                        nc.sync.dma_start(out_d[h, :, 2048 * a:2048 * a + 2048],
                                          stag[h][:, 2048 * a:2048 * a + 2048])

    nc.compile()
    return nc


def _prep_inputs(inputs):
    hs = np.asarray(inputs["hidden_states"], dtype=np.float32)
    maps = []
    j = np.arange(KB)
    p = np.arange(KB)[:, None]
    m_lo = np.where(j[None, :] >= p, 0.0, NEG).astype(np.float32)
    m_hi = np.where(j[None, :] <= p, 0.0, NEG).astype(np.float32)
    masks = np.concatenate([m_lo, m_hi], axis=1)
    id96 = np.eye(96, dtype=bf16)

    def wtiles(w):
        n = w.shape[1]
        return np.ascontiguousarray(w.reshape(NKT, 128, n).transpose(1, 0, 2)).astype(bf16)

    for c in range(8):
        b, hg = c // 4, c % 4
        cols = slice(192 * hg, 192 * hg + 192)
        Wq = np.asarray(inputs["Wq"], np.float32)[:, cols] * SCALE
        bq = np.asarray(inputs["bq"], np.float32)[cols] * SCALE
        Wqg = np.asarray(inputs["Wqg"], np.float32)[:, cols] * SCALE
        bqg = np.asarray(inputs["bqg"], np.float32)[cols] * SCALE
        Wk = np.asarray(inputs["Wk"], np.float32)[:, cols]
        bk = np.asarray(inputs["bk"], np.float32)[cols]
        Wkg = np.asarray(inputs["Wkg"], np.float32)[:, cols]
        bkg = np.asarray(inputs["bkg"], np.float32)[cols]
        Wv = np.asarray(inputs["Wv"], np.float32)[:, cols]
        bv = np.asarray(inputs["bv"], np.float32)[cols]
        Wvg = np.asarray(inputs["Wvg"], np.float32)[:, cols]
        bvg = np.asarray(inputs["bvg"], np.float32)[cols]

        # m2 biases live at partitions 64-127 now
        bias_t2 = np.zeros((128, 8), np.float32)
        bias_t2[:, 0], bias_t2[64:128, 1] = bq[0:128], bq[128:192]
        bias_t2[:, 2], bias_t2[64:128, 3] = bk[0:128], bk[128:192]
        bias_t2[:, 4], bias_t2[64:128, 5] = bkg[0:128], bkg[128:192]
        bias_t2[:, 6], bias_t2[64:128, 7] = bqg[0:128], bqg[128:192]

        maps.append({
            "hsb": hs[b].astype(bf16),
            "w_q": wtiles(Wq),
            "w_k": wtiles(Wk),
            "w_kg": wtiles(Wkg),
            "w_qg": wtiles(Wqg),
            "w_vvg": wtiles(np.concatenate([Wv, Wvg], axis=1)),
            "bvvg": np.concatenate([bv, bvg])[None, :].astype(bf16),
            "bias_t": bias_t2,
            "masks": masks,
            "id96": id96,
        })
    return maps


def kernel(**inputs):
    g = int(np.asarray(inputs["num_global"]))
    assert g == G, f"kernel compiled for num_global=32, got {g}"
    if "nc" not in _cache:
        _cache["nc"] = _build()
    nc = _cache["nc"]
    in_maps = _prep_inputs(inputs)
    res = run_bass_kernel_spmd(nc, in_maps, list(range(8)))
    return assemble(res.results)


def assemble(results):
    out = np.zeros((B, S, D), np.float32)
    for c in range(8):
        b, hg = c // 4, c % 4
        o = results[c]["out"]          # [3, 65, S]
        og = results[c]["outg"]        # [G, 3, 65]
        for h in range(3):
            col = 192 * hg + 64 * h
            out[b, :, col:col + 64] = (o[h, 0:64] / o[h, 64]).T
            out[b, 0:G, col:col + 64] = og[:, h, 0:64] / og[:, h, 64:65]
    return out


# revision 16
# speedup vs baseline: 1.5037x; 1.2528x over previous
"""Trainium2 Bass kernel for Longformer self-attention (B=2, S=4096, D=768, H=12, HD=64, W=256, G=32).

Sharding: 8 cores = 2 batches x 4 head-groups (3 heads each). Each core computes its
batch's projections restricted to its 192 output channels, runs banded + global
attention for its 3 heads, and returns an unnormalized transposed output
([3, 65, S]: rows 0-63 = head-dim, row 64 = softmax denominator z) plus the raw
global-query output; the host divides by z, transposes, and assembles.

On-chip layout (per core):
  qT/kT/kgT  : transposed projections, head-dim on partitions. Heads 0/1 share a
               [128, S] tile (rows 0-63 / 64-127); head 2 occupies rows 64-127 of a
               second tile so its matmuls sit in PE row-groups 2-3 and run
               concurrently with head-0 work (row-group packing).
  v/vg       : natural layout [128-part seq tiles, per-head 64 cols + ones column]
  band scores: computed transposed ([128 keys, 640-query span] per key block), so
               the softmax denominator comes free via the ones column in PV.
Matmul inputs bf16 (hidden_states pre-cast on host), fp32 PSUM/softmax.
"""
import numpy as np
import ml_dtypes

import concourse.bass as bass
import concourse.mybir as mybir
import concourse.tile as tile
from concourse import bacc
from concourse.bass_utils import run_bass_kernel_spmd

B, S, D, H, HD = 2, 4096, 768, 12, 64
W = 256
G = 32
SCALE = 1.0 / np.float32(np.sqrt(HD))
NEG = -60.0
KB = 128
NKB = S // KB     # 32
QSB = 512
NQSB = S // QSB   # 8
NKT = D // 128    # 6
NNT = S // 512    # 8

BF = mybir.dt.bfloat16
F32 = mybir.dt.float32
AF = mybir.ActivationFunctionType
bf16 = ml_dtypes.bfloat16

_cache = {}


def _span(kb):
    k0 = KB * kb
    qlo, qhi = max(0, k0 - 2 * KB), min(S, k0 + 3 * KB)
    return qlo, qhi, qlo - (k0 - 2 * KB), qhi - (k0 - 2 * KB)


def _build():
    nc = bacc.Bacc(None, target_bir_lowering=False)

    hsb = nc.declare_dram_parameter("hsb", [S, D], BF, isOutput=False)
    w_q = nc.declare_dram_parameter("w_q", [128, NKT, 192], BF, isOutput=False)
    w_k = nc.declare_dram_parameter("w_k", [128, NKT, 192], BF, isOutput=False)
    w_kg = nc.declare_dram_parameter("w_kg", [128, NKT, 192], BF, isOutput=False)
    w_qg = nc.declare_dram_parameter("w_qg", [128, NKT, 192], BF, isOutput=False)
    w_vvg = nc.declare_dram_parameter("w_vvg", [128, NKT, 384], BF, isOutput=False)
    bvvg_d = nc.declare_dram_parameter("bvvg", [1, 384], BF, isOutput=False)
    bias_d = nc.declare_dram_parameter("bias_t", [128, 8], F32, isOutput=False)
    masks_d = nc.declare_dram_parameter("masks", [128, 256], F32, isOutput=False)
    id96_d = nc.declare_dram_parameter("id96", [96, 96], BF, isOutput=False)
    out_d = nc.declare_dram_parameter("out", [3, 65, S], F32, isOutput=True)
    outg_d = nc.declare_dram_parameter("outg", [G, 3, 65], F32, isOutput=True)

    with tile.TileContext(nc) as tc:
        with tc.tile_pool(name="persist", bufs=1) as pp:
            masks_t = pp.tile([128, 256], F32)
            ones_t = pp.tile([1, 128], BF)
            nc.sync.dma_start(masks_t[:], masks_d[:])
            nc.vector.memset(ones_t[:], 1.0)

            qT01 = pp.tile([128, S], BF)
            qT2 = pp.tile([128, S], BF)      # rows 64-127 used
            kT01 = pp.tile([128, S], BF)
            kT2 = pp.tile([128, S], BF)      # rows 64-127 used
            v_nat = pp.tile([128, NKB, 3, 65], BF)
            exp_sg = pp.tile([G, 3, S], BF)
            nc.vector.memset(v_nat[:, :, :, 64:65], 1.0)

            def sl(t01, t2, h):
                return t01[64 * h:64 * h + 64] if h < 2 else t2[64:128]

            with tc.tile_pool(name="ac", bufs=1) as ac:
                kgT01 = ac.tile([128, S], BF)
                kgT2 = ac.tile([128, S], BF)  # rows 64-127 used
                qgT01 = ac.tile([128, G], BF)
                qgT2 = ac.tile([128, G], BF)  # rows 64-127 used
                vg_nat = ac.tile([128, NKB, 3, 65], BF)
                probs_g = ac.tile([96, S], BF)
                pb_gT = ac.tile([128, NKB, 96], BF)
                id96_t = ac.tile([96, 96], BF)
                og_sb = ac.tile([G, 3, 65], F32)
                nc.vector.memset(vg_nat[:, :, :, 64:65], 1.0)

                # ---------------- Phase A: projections --------------------------
                with (
                    tc.tile_pool(name="aw", bufs=1) as aw,
                    tc.tile_pool(name="hst", bufs=3) as hstp,
                    tc.tile_pool(name="apsum", bufs=3, space="PSUM") as apsum,
                    tc.tile_pool(name="spsum", bufs=2, space="PSUM") as spsum,
                    tc.tile_pool(name="opsum", bufs=1, space="PSUM") as opsum,
                    tc.tile_pool(name="pbt", bufs=26) as pbtp,
                    tc.tile_pool(name="osb", bufs=3) as osbp,
                ):
                    wq_t = aw.tile([128, NKT, 192], BF)
                    wk_t = aw.tile([128, NKT, 192], BF)
                    wkg_t = aw.tile([128, NKT, 192], BF)
                    wqg_t = aw.tile([128, NKT, 192], BF)
                    wvvg_t = aw.tile([128, NKT, 384], BF)
                    bvvg_t = aw.tile([1, 384], BF)
                    bias_t = aw.tile([128, 8], F32)
                    nc.sync.dma_start(wq_t[:], w_q[:])
                    nc.sync.dma_start(wk_t[:], w_k[:])
                    nc.sync.dma_start(wkg_t[:], w_kg[:])
                    nc.sync.dma_start(wqg_t[:], w_qg[:])
                    nc.sync.dma_start(wvvg_t[:], w_vvg[:])
                    nc.sync.dma_start(bvvg_t[:], bvvg_d[:])
                    nc.sync.dma_start(bias_t[:], bias_d[:])
                    nc.sync.dma_start(id96_t[:], id96_d[:])

                    tproj = [
                        (wq_t, 0, 1, qT01, qT2),
                        (wk_t, 2, 3, kT01, kT2),
                        (wkg_t, 4, 5, kgT01, kgT2),
                    ]
                    pbt = {}

                    def do_kb(kb):
                        k0 = KB * kb
                        qlo, qhi, llo, lhi = _span(kb)
                        pieces = [(a, b) for (a, b) in
                                  [(llo, min(lhi, 512)), (max(llo, 512), lhi)] if a < b]
                        tiles = []
                        for h in range(3):
                            tiles.append(spsum.tile([128, 640], F32, tag="sc", name=f"sc{kb}_{h}"))
                        for (a, b2) in pieces:
                            for h in range(3):
                                nc.tensor.matmul(
                                    tiles[h][:, a:b2],
                                    sl(kT01, kT2, h)[:, k0:k0 + KB],
                                    sl(qT01, qT2, h)[:, qlo + (a - llo):qlo + (a - llo) + (b2 - a)])
                        for h in range(3):
                            ps = tiles[h]
                            if llo == 0:
                                nc.vector.tensor_add(ps[:, 0:KB], ps[:, 0:KB], masks_t[:, 0:KB])
                            if lhi == 5 * KB:
                                nc.vector.tensor_add(ps[:, 512:640], ps[:, 512:640],
                                                     masks_t[:, KB:2 * KB])
                            t_ = pbtp.tile([128, 640], BF, tag="pb")
                            nc.scalar.activation(t_[:, llo:lhi], ps[:, llo:lhi], AF.Exp)
                            pbt[(kb, h)] = t_

                    def do_pv(qs):
                        q0 = QSB * qs
                        kbs = list(range(max(0, 4 * qs - 2), min(NKB, 4 * qs + 6)))
                        for h in range(3):
                            po = opsum.tile([65, 512], F32, tag="po")
                            nc.tensor.matmul(po[:], v_nat[0:G, 0, h, :],
                                             exp_sg[:, h, q0:q0 + 512],
                                             start=True, stop=False)
                            for i, kb in enumerate(kbs):
                                k0 = KB * kb
                                qlo, qhi, llo, lhi = _span(kb)
                                a, b2 = max(qlo, q0), min(qhi, q0 + QSB)
                                la = a - (k0 - 2 * KB)
                                nc.tensor.matmul(po[:, a - q0:b2 - q0], v_nat[:, kb, h, :],
                                                 pbt[(kb, h)][:, la:la + (b2 - a)],
                                                 start=False, stop=(i == len(kbs) - 1))
                            ob = osbp.tile([65, 512], F32, tag="ob")
                            if h % 2 == 0:
                                nc.vector.tensor_copy(ob[:], po[:])
                            else:
                                nc.scalar.copy(ob[:], po[:])
                            nc.sync.dma_start(out_d[h, :, q0:q0 + 512], ob[:])

                    emitted_kb = 0
                    emitted_qs = 0
                    for nt in range(NNT):
                        c0 = 512 * nt
                        hst = hstp.tile([128, NKT, 512], BF)
                        for kt in range(NKT):
                            eng = nc.sync
                            eng.dma_start(
                                out=hst[:, kt, :],
                                in_=hsb[c0:c0 + 512, 128 * kt:128 * kt + 128],
                                transpose=True,
                            )
                        for (wt, c01, c2, d01, d2) in tproj:
                            ps = apsum.tile([128, 512], F32, tag="pp")
                            for kt in range(NKT):
                                nc.tensor.matmul(ps[:], wt[:, kt, 0:128], hst[:, kt, :],
                                                 start=(kt == 0), stop=(kt == NKT - 1))
                            nc.vector.tensor_scalar_add(
                                d01[:, c0:c0 + 512], ps[:], bias_t[:, c01:c01 + 1])
                            ps2 = apsum.tile([128, 512], F32, tag="pp")
                            for kt in range(NKT):
                                nc.tensor.matmul(ps2[64:128, :], wt[:, kt, 128:192],
                                                 hst[:, kt, :],
                                                 start=(kt == 0), stop=(kt == NKT - 1))
                            nc.scalar.activation(
                                d2[64:128, c0:c0 + 512], ps2[64:128, :], AF.Identity,
                                bias=bias_t[64:128, c2:c2 + 1], scale=1.0)
                        for s4 in range(4):
                            sb = 4 * nt + s4
                            psv = apsum.tile([128, 384], F32, tag="pp")
                            for kt in range(NKT):
                                nc.tensor.matmul(psv[:], hst[:, kt, 128 * s4:128 * s4 + 128],
                                                 wvvg_t[:, kt, :],
                                                 start=(kt == 0), stop=False)
                            nc.tensor.matmul(psv[:], ones_t[:, 0:128], bvvg_t[:],
                                             start=False, stop=True)
                            nc.vector.tensor_copy(
                                v_nat[:, sb, :, 0:64],
                                psv[:, 0:192].rearrange("p (h e) -> p h e", h=3))
                            nc.scalar.copy(
                                vg_nat[:, sb, :, 0:64],
                                psv[:, 192:384].rearrange("p (h e) -> p h e", h=3))
                        if nt == 0:
                            psq = apsum.tile([128, G], F32, tag="pp")
                            for kt in range(NKT):
                                nc.tensor.matmul(psq[:], wqg_t[:, kt, 0:128],
                                                 hst[:, kt, 0:G],
                                                 start=(kt == 0), stop=(kt == NKT - 1))
                            nc.vector.tensor_scalar_add(qgT01[:], psq[:], bias_t[:, 6:7])
                            psq2 = apsum.tile([128, G], F32, tag="pp")
                            for kt in range(NKT):
                                nc.tensor.matmul(psq2[64:128, :], wqg_t[:, kt, 128:192],
                                                 hst[:, kt, 0:G],
                                                 start=(kt == 0), stop=(kt == NKT - 1))
                            nc.vector.tensor_scalar_add(qgT2[64:128, :], psq2[64:128, :],
                                                        bias_t[64:128, 7:8])
                        for h in range(3):
                            ps_sg = apsum.tile([G, 512], F32, tag="pp", name=f"sg{nt}_{h}")
                            nc.tensor.matmul(ps_sg[:],
                                             sl(kT01, kT2, h)[:, 0:G],
                                             sl(qT01, qT2, h)[:, c0:c0 + 512])
                            nc.scalar.activation(exp_sg[:, h, c0:c0 + 512], ps_sg[:], AF.Exp)
                        while emitted_kb <= min(4 * nt + 1, NKB - 1):
                            do_kb(emitted_kb)
                            emitted_kb += 1
                        while emitted_qs < NQSB and 4 * emitted_qs + 5 <= emitted_kb - 1 \
                                and emitted_qs <= nt - 1:
                            do_pv(emitted_qs)
                            emitted_qs += 1
                    while emitted_kb < NKB:
                        do_kb(emitted_kb)
                        emitted_kb += 1
                    while emitted_qs < NQSB:
                        do_pv(emitted_qs)
                        emitted_qs += 1

                # ------------- Phase C: global-query attention ------------------
                with (
                    tc.tile_pool(name="cpsum", bufs=2, space="PSUM") as cpsum,
                    tc.tile_pool(name="cgps", bufs=3, space="PSUM") as cgps,
                ):
                    for c8 in range(NNT):
                        c0 = 512 * c8
                        ps = cpsum.tile([96, 512], F32, tag="qg")
                        for h in range(3):
                            nc.tensor.matmul(ps[32 * h:32 * h + 32, :],
                                             sl(qgT01, qgT2, h)[:],
                                             sl(kgT01, kgT2, h)[:, c0:c0 + 512])
                        nc.scalar.activation(probs_g[:, c0:c0 + 512], ps[:], AF.Exp)
                    for t in range(NKB):
                        pst = cpsum.tile([128, 96], BF, tag="qg")
                        nc.tensor.transpose(pst[:], probs_g[:, 128 * t:128 * t + 128],
                                            id96_t[:])
                        if t % 2 == 0:
                            nc.vector.tensor_copy(pb_gT[:, t, :], pst[:])
                        else:
                            nc.scalar.copy(pb_gT[:, t, :], pst[:])
                    for h in range(3):
                        go = cgps.tile([G, 65], F32, tag="go")
                        for t in range(NKB):
                            nc.tensor.matmul(go[:], pb_gT[:, t, 32 * h:32 * h + 32],
                                             vg_nat[:, t, h, :],
                                             start=(t == 0), stop=(t == NKB - 1))
                        nc.vector.tensor_copy(og_sb[:, h, :], go[:])
                    nc.sync.dma_start(outg_d[:], og_sb[:])

    nc.compile()
    return nc


def _prep_inputs(inputs):
    hs = np.asarray(inputs["hidden_states"], dtype=np.float32)
    maps = []
    j = np.arange(KB)
    p = np.arange(KB)[:, None]
    m_lo = np.where(j[None, :] >= p, 0.0, NEG).astype(np.float32)
    m_hi = np.where(j[None, :] <= p, 0.0, NEG).astype(np.float32)
    masks = np.concatenate([m_lo, m_hi], axis=1)
    id96 = np.eye(96, dtype=bf16)

    def wtiles(w):
        n = w.shape[1]
        return np.ascontiguousarray(w.reshape(NKT, 128, n).transpose(1, 0, 2)).astype(bf16)

    for c in range(8):
        b, hg = c // 4, c % 4
        cols = slice(192 * hg, 192 * hg + 192)
        Wq = np.asarray(inputs["Wq"], np.float32)[:, cols] * SCALE
        bq = np.asarray(inputs["bq"], np.float32)[cols] * SCALE
        Wqg = np.asarray(inputs["Wqg"], np.float32)[:, cols] * SCALE
        bqg = np.asarray(inputs["bqg"], np.float32)[cols] * SCALE
        Wk = np.asarray(inputs["Wk"], np.float32)[:, cols]
        bk = np.asarray(inputs["bk"], np.float32)[cols]
        Wkg = np.asarray(inputs["Wkg"], np.float32)[:, cols]
        bkg = np.asarray(inputs["bkg"], np.float32)[cols]
        Wv = np.asarray(inputs["Wv"], np.float32)[:, cols]
        bv = np.asarray(inputs["bv"], np.float32)[cols]
        Wvg = np.asarray(inputs["Wvg"], np.float32)[:, cols]
        bvg = np.asarray(inputs["bvg"], np.float32)[cols]

        # m2 biases live at partitions 64-127 now
        bias_t2 = np.zeros((128, 8), np.float32)
        bias_t2[:, 0], bias_t2[64:128, 1] = bq[0:128], bq[128:192]
        bias_t2[:, 2], bias_t2[64:128, 3] = bk[0:128], bk[128:192]
        bias_t2[:, 4], bias_t2[64:128, 5] = bkg[0:128], bkg[128:192]
        bias_t2[:, 6], bias_t2[64:128, 7] = bqg[0:128], bqg[128:192]

        maps.append({
            "hsb": hs[b].astype(bf16),
            "w_q": wtiles(Wq),
            "w_k": wtiles(Wk),
            "w_kg": wtiles(Wkg),
            "w_qg": wtiles(Wqg),
            "w_vvg": wtiles(np.concatenate([Wv, Wvg], axis=1)),
            "bvvg": np.concatenate([bv, bvg])[None, :].astype(bf16),
            "bias_t": bias_t2,
            "masks": masks,
            "id96": id96,
        })
    return maps


def kernel(**inputs):
    g = int(np.asarray(inputs["num_global"]))
    assert g == G, f"kernel compiled for num_global=32, got {g}"
    if "nc" not in _cache:
        _cache["nc"] = _build()
    nc = _cache["nc"]
    in_maps = _prep_inputs(inputs)
    res = run_bass_kernel_spmd(nc, in_maps, list(range(8)))
    return assemble(res.results)


def assemble(results):
    out = np.zeros((B, S, D), np.float32)
    for c in range(8):
        b, hg = c // 4, c % 4
        o = results[c]["out"]          # [3, 65, S]
        og = results[c]["outg"]        # [G, 3, 65]
        for h in range(3):
            col = 192 * hg + 64 * h
            out[b, :, col:col + 64] = (o[h, 0:64] / o[h, 64]).T
            out[b, 0:G, col:col + 64] = og[:, h, 0:64] / og[:, h, 64:65]
    return out
